# revision 2
# baseline (speedup 1.0000x reference)
"""Trainium2 Bass kernel for nn_ACAClassifier (soft cellular-automaton update).

Reference computation, per depth d (8 depths):
    mask = sigmoid(state @ W[d].T + b[d])
    t    = 4*l + 2*c + r          (circular neighbors along feature axis)
    nb   = int(t)                 (truncation)
    bits = rule110_table[7 - nb]  == [t>=1] - [t>=4] + [t>=5] - [t>=7]
    state = mask*bits + (1-mask)*state

Sharding: pure data-parallel over the batch axis across 8 NeuronCores.

Per-core layout (natural layout, batch on partitions):
  - state tiles [128, G, 514] resident in SBUF per group of G tiles,
    columns 0/513 are circular-wrap duplicates so l/c/r are plain slices.
  - mask matmul: PE-transpose state chunks -> state_T (stationary), W.T
    chunks (host-pretransposed) as moving operand -> psum in natural layout.
  - neighborhood t: either on DVE (exact fp32, matches reference rounding
    order) or on PE via a banded circulant matmul.
  - bits via fp32 compares on DVE (exact 0/1), blend in fp32.
"""

import sys

for _p in ("/opt/pypackages", "/opt/trn_rl_repo"):
    if _p not in sys.path:
        sys.path.insert(0, _p)

import numpy as np

BATCH = 65536
SIZE = 512
DEPTH = 8
N_CORES = 8
ROWS_PER_CORE = BATCH // N_CORES  # 8192
NTILES = ROWS_PER_CORE // 128     # 64

_NC_CACHE = {}


def build_nc(ntiles, G, with_bias=False, t_on_pe=False, mm_f32r=False,
             num_devices=N_CORES, state_bufs=6, tmp_bufs=3, st_bufs=4):
    """Build + compile the per-core Bass program.

    ntiles: number of 128-row batch tiles per core.
    G: tiles per group (DVE/ScalarE instructions batch over a group).
    """
    import concourse.bacc as bacc
    import concourse.mybir as mybir
    import concourse.tile as tile

    assert ntiles % G == 0
    ngroups = ntiles // G
    f32 = mybir.dt.float32
    f32r = mybir.dt.float32r
    bf16 = mybir.dt.bfloat16
    AL = mybir.AluOpType
    AF = mybir.ActivationFunctionType

    def mm_cast(ap):
        return ap.bitcast(f32r) if mm_f32r else ap

    nc = bacc.Bacc("TRN2", target_bir_lowering=False, debug=False,
                   num_devices=num_devices)
    rows = ntiles * 128
    x_d = nc.dram_tensor("x", [rows, SIZE], f32, kind="ExternalInput")
    wt_d = nc.dram_tensor("wt", [128, DEPTH, 4, SIZE], f32, kind="ExternalInput")
    id_d = nc.dram_tensor("ident", [128, 128], f32, kind="ExternalInput")
    if t_on_pe:
        ct_d = nc.dram_tensor("ct", [128, 130], f32, kind="ExternalInput")
    if with_bias:
        ones_d = nc.dram_tensor("ones", [1, 128], f32, kind="ExternalInput")
        b_d = nc.dram_tensor("b", [1, DEPTH * SIZE], f32, kind="ExternalInput")
    out_d = nc.dram_tensor("out", [rows, SIZE], f32, kind="ExternalOutput")

    with tile.TileContext(nc) as tc:
        with (
            tc.tile_pool(name="const", bufs=1) as constp,
            tc.tile_pool(name="state", bufs=state_bufs) as statep,
            tc.tile_pool(name="tmpf", bufs=tmp_bufs) as tmpf,
            tc.tile_pool(name="tmpb", bufs=tmp_bufs) as tmpb,
            tc.tile_pool(name="stp", bufs=st_bufs) as stp,
            tc.tile_pool(name="psA", bufs=2, space="PSUM") as psA,
            tc.tile_pool(name="psM", bufs=2, space="PSUM") as psM,
            tc.tile_pool(name="psT2", bufs=2, space="PSUM") as psT2,
        ):
            wt_sb = constp.tile([128, DEPTH, 4, SIZE], f32, tag="wt")
            nc.sync.dma_start(wt_sb[:], wt_d.ap())
            id_sb = constp.tile([128, 128], f32, tag="id")
            nc.sync.dma_start(id_sb[:], id_d.ap())
            if t_on_pe:
                ct_sb = constp.tile([128, 130], f32, tag="ct")
                nc.sync.dma_start(ct_sb[:], ct_d.ap())
            if with_bias:
                ones_sb = constp.tile([1, 128], f32, tag="ones")
                nc.sync.dma_start(ones_sb[:], ones_d.ap())
                b_sb = constp.tile([1, DEPTH * SIZE], f32, tag="b")
                nc.sync.dma_start(b_sb[:], b_d.ap())

            x_ap = x_d.ap()
            out_ap = out_d.ap()
            for g in range(ngroups):
                st = statep.tile([128, G, SIZE + 2], f32, tag="st")
                for i in range(G):
                    r0 = (g * G + i) * 128
                    nc.sync.dma_start(st[:, i, 1:SIZE + 1], x_ap[r0:r0 + 128, :])
                nc.vector.tensor_copy(st[:, :, 0:1], st[:, :, SIZE:SIZE + 1])
                nc.vector.tensor_copy(st[:, :, SIZE + 1:SIZE + 2], st[:, :, 1:2])

                for d in range(DEPTH):
                    pM = psM.tile([128, G, SIZE], f32, tag="pM")
                    mask = tmpf.tile([128, G, SIZE], f32, tag="mask")
                    if t_on_pe:
                        pT2 = psT2.tile([128, G, SIZE], f32, tag="pT2")
                    for i in range(G):
                        sT = stp.tile([128, SIZE], f32, tag="sT")
                        pT = psA.tile([128, SIZE], f32, tag="pT")
                        for j in range(4):
                            nc.tensor.transpose(
                                pT[:, j * 128:(j + 1) * 128],
                                st[:, i, 1 + j * 128:1 + (j + 1) * 128],
                                id_sb[:],
                            )
                        nc.scalar.copy(sT[:], pT[:])
                        for j in range(4):
                            nc.tensor.matmul(
                                pM[:, i, :],
                                mm_cast(sT[:, j * 128:(j + 1) * 128]),
                                mm_cast(wt_sb[:, d, j, :]),
                                start=(j == 0),
                                stop=(j == 3 and not with_bias),
                            )
                        if with_bias:
                            nc.tensor.matmul(
                                pM[:, i, :],
                                ones_sb[0:1, :],
                                b_sb[0:1, d * SIZE:(d + 1) * SIZE],
                                start=False,
                                stop=True,
                            )
                        if t_on_pe:
                            # banded circulant: t = 4*l + 2*c + r on PE.
                            # chunk j's matmul covers out cols
                            # [128j-1, 128j+128]; wrap contributions via two
                            # 1-column corner matmuls, ordered after the
                            # chunk matmuls so the fp32 rounding order
                            # matches the reference where possible.
                            nc.tensor.matmul(
                                pT2[:, i, 0:129], sT[:, 0:128], ct_sb[:, 1:130],
                                start=True, stop=False, skip_group_check=True)
                            for j in (1, 2):
                                nc.tensor.matmul(
                                    pT2[:, i, j * 128 - 1:j * 128 + 129],
                                    sT[:, j * 128:(j + 1) * 128],
                                    ct_sb[:, 0:130],
                                    start=False, stop=False,
                                    skip_group_check=True)
                            nc.tensor.matmul(
                                pT2[:, i, 383:SIZE], sT[:, 384:SIZE],
                                ct_sb[:, 0:129],
                                start=False, stop=False, skip_group_check=True)
                            # corners: t[511] += 1*s[0]; t[0] += 4*s[511]
                            nc.tensor.matmul(
                                pT2[:, i, 511:512], sT[:, 0:128], ct_sb[:, 0:1],
                                start=False, stop=False, skip_group_check=True)
                            nc.tensor.matmul(
                                pT2[:, i, 0:1], sT[:, 384:SIZE],
                                ct_sb[:, 129:130],
                                start=False, stop=True, skip_group_check=True)
                    nc.scalar.activation(mask[:], pM[:], AF.Sigmoid)

                    l_ap = st[:, :, 0:SIZE]
                    c_ap = st[:, :, 1:SIZE + 1]
                    r_ap = st[:, :, 2:SIZE + 2]
                    tb = tmpf.tile([128, G, SIZE], f32, tag="tb")
                    if t_on_pe:
                        t_src = pT2[:]
                    else:
                        c2 = tmpf.tile([128, G, SIZE], f32, tag="c2")
                        # u = 4*l + 2*c (both scalings exact, one add)
                        nc.vector.tensor_scalar(c2[:], c_ap, 2.0, None, AL.mult)
                        nc.vector.scalar_tensor_tensor(
                            c2[:], l_ap, 4.0, c2[:], AL.mult, AL.add)
                        # t = u + r
                        nc.vector.tensor_tensor(tb[:], c2[:], r_ap, AL.add)
                        t_src = tb[:]
                    g4 = tmpb.tile([128, G, SIZE], bf16, tag="g4")
                    g7 = tmpb.tile([128, G, SIZE], bf16, tag="g7")
                    nc.vector.tensor_scalar(g4[:], t_src, 4.0, None, AL.is_ge)
                    nc.vector.tensor_scalar(g7[:], t_src, 7.0, None, AL.is_ge)
                    # d1 = [t>=1] - [t>=4]; d2 = [t>=5] - [t>=7]
                    nc.vector.scalar_tensor_tensor(
                        g4[:], t_src, 1.0, g4[:], AL.is_ge, AL.subtract)
                    nc.vector.scalar_tensor_tensor(
                        g7[:], t_src, 5.0, g7[:], AL.is_ge, AL.subtract)
                    # bits = d1 + d2  (exact 0/1 in bf16)
                    nc.vector.tensor_tensor(g4[:], g4[:], g7[:], AL.add)
                    # e = bits - state
                    nc.vector.scalar_tensor_tensor(
                        tb[:], c_ap, -1.0, g4[:], AL.mult, AL.add)
                    # f = mask * e
                    nc.vector.tensor_tensor(tb[:], mask[:], tb[:], AL.mult)
                    # state += f   (in-place)
                    nc.vector.tensor_tensor(st[:, :, 1:SIZE + 1], tb[:], c_ap,
                                            AL.add)
                    # refresh circular-wrap pad columns
                    nc.vector.tensor_copy(st[:, :, 0:1], st[:, :, SIZE:SIZE + 1])
                    nc.vector.tensor_copy(st[:, :, SIZE + 1:SIZE + 2],
                                          st[:, :, 1:2])

                for i in range(G):
                    r0 = (g * G + i) * 128
                    nc.sync.dma_start(out_ap[r0:r0 + 128, :], st[:, i, 1:SIZE + 1])

    nc.compile()
    return nc


def _host_inputs(W, b, with_bias, t_on_pe):
    W = np.asarray(W, dtype=np.float32)
    # wt[p, d, j, n] = W[d][n, j*128+p]
    wt = np.ascontiguousarray(
        W.transpose(0, 2, 1).reshape(DEPTH, 4, 128, SIZE).transpose(2, 0, 1, 3))
    common = {
        "wt": wt,
        "ident": np.eye(128, dtype=np.float32),
    }
    if t_on_pe:
        # ct[k, m]: coeff of s_k in window col m (global out col 128j-1+m):
        # t_n = 4 s_{n-1} + 2 s_n + s_{n+1}
        ct = np.zeros((128, 130), dtype=np.float32)
        for k in range(128):
            ct[k, k] = 1.0
            ct[k, k + 1] = 2.0
            ct[k, k + 2] = 4.0
        common["ct"] = ct
    if with_bias:
        common["ones"] = np.ones((1, 128), dtype=np.float32)
        common["b"] = np.ascontiguousarray(
            np.asarray(b, dtype=np.float32).reshape(1, DEPTH * SIZE))
    return common


# default configuration used by kernel()
CFG = dict(G=2, t_on_pe=False, mm_f32r=False)


def get_nc(with_bias, cfg=None):
    cfg = dict(CFG if cfg is None else cfg)
    key = (NTILES, with_bias, tuple(sorted(cfg.items())))
    if key not in _NC_CACHE:
        _NC_CACHE[key] = build_nc(NTILES, cfg["G"], with_bias=with_bias,
                                  t_on_pe=cfg["t_on_pe"],
                                  mm_f32r=cfg["mm_f32r"])
    return _NC_CACHE[key]


def make_in_maps(x, W, b, with_bias, cfg=None):
    cfg = dict(CFG if cfg is None else cfg)
    common = _host_inputs(W, b, with_bias, cfg["t_on_pe"])
    shards = np.asarray(x, dtype=np.float32).reshape(-1, ROWS_PER_CORE, SIZE)
    return [dict(common, x=np.ascontiguousarray(shards[i]))
            for i in range(shards.shape[0])]


def kernel(x, W, b):
    from concourse import bass_utils

    x = np.asarray(x, dtype=np.float32)
    b = np.asarray(b, dtype=np.float32)
    assert x.shape == (BATCH, SIZE)
    with_bias = bool(np.any(b))
    nc = get_nc(with_bias)
    in_maps = make_in_maps(x, W, b, with_bias)
    res = bass_utils.run_bass_kernel_spmd(nc, in_maps,
                                          core_ids=list(range(N_CORES)))
    out = np.concatenate([res.results[i]["out"] for i in range(N_CORES)], axis=0)
    return out.astype(np.float32, copy=False)


# revision 26
# speedup vs baseline: 1814.0099x; 1814.0099x over previous
"""Trainium2 Bass kernel for nn_ACAClassifier (soft cellular-automaton update).

Reference computation, per depth d (8 depths):
    mask = sigmoid(state @ W[d].T + b[d])
    t    = 4*l + 2*c + r          (circular neighbors along feature axis)
    nb   = int(t)                 (truncation)
    bits = rule110_table[7 - nb]  == [t>=1] - [t>=4] + [t>=5] - [t>=7]
    state = mask*bits + (1-mask)*state

Sharding: pure data-parallel over the batch axis across 8 NeuronCores.

Per-core structure (depth-outer so all engines pipeline across groups):
  - all 64 batch tiles stay SBUF-resident as [128, G, 514] group tiles
    (columns 0/513 are circular-wrap duplicates so l/c/r are plain slices);
    W.T is streamed from HBM one depth at a time (double-buffered).
  - mask matmul: PE-transpose state chunks -> state_T (stationary), W.T
    chunks (host-pretransposed) as moving operand -> psum, natural layout.
  - neighborhood t: on DVE (exact fp32, matches reference rounding order)
    or on PE via a banded circulant matmul (t_on_pe).
  - bits via fp32 compares on DVE (exact 0/1), fp32 blend.
"""

import sys

for _p in ("/opt/pypackages", "/opt/trn_rl_repo"):
    if _p not in sys.path:
        sys.path.insert(0, _p)

import numpy as np

BATCH = 65536
SIZE = 512
DEPTH = 8
N_CORES = 8
ROWS_PER_CORE = BATCH // N_CORES  # 8192
NTILES = ROWS_PER_CORE // 128     # 64

_NC_CACHE = {}


def build_nc(ntiles, G=2, with_bias=False, t_on_pe=False, mm_pair=False,
             abs_on_sc=False, c2_on_sc=False,
             num_devices=N_CORES, tmp_bufs=2, mask_bufs=3, st_bufs=4,
             repeat=1, skip_ew=False, skip_mm=False):
    """Build + compile the per-core Bass program (depth-outer schedule).

    mm_pair: 3-term bf16-pair mask matmul (s_hi@w_hi + s_hi@w_lo + s_lo@w_hi)
             instead of native fp32 (4 cyc/row -> 3x 1 cyc/row on PE).
    abs_on_sc: compute r1=|t-2.5|, r2=|t-6| on ScalarE; bits via is_lt on DVE.
    c2_on_sc: compute 2*c on ScalarE instead of DVE.
    """
    import concourse.bacc as bacc
    import concourse.mybir as mybir
    import concourse.tile as tile

    assert ntiles % G == 0
    ngroups = ntiles // G
    f32 = mybir.dt.float32
    bf16 = mybir.dt.bfloat16
    AL = mybir.AluOpType
    AF = mybir.ActivationFunctionType

    nc = bacc.Bacc("TRN2", target_bir_lowering=False, debug=False,
                   num_devices=num_devices)
    rows = ntiles * 128
    x_d = nc.dram_tensor("x", [rows, SIZE], f32, kind="ExternalInput")
    if mm_pair:
        wth_d = nc.dram_tensor("wt_hi", [128, DEPTH, 4, SIZE], bf16,
                               kind="ExternalInput")
        wtl_d = nc.dram_tensor("wt_lo", [128, DEPTH, 4, SIZE], bf16,
                               kind="ExternalInput")
    else:
        wt_d = nc.dram_tensor("wt", [128, DEPTH, 4, SIZE], f32,
                              kind="ExternalInput")
    id_d = nc.dram_tensor("ident", [128, 128], f32, kind="ExternalInput")
    if t_on_pe:
        ct_d = nc.dram_tensor("ct", [128, 130], f32, kind="ExternalInput")
    if with_bias:
        ones_d = nc.dram_tensor("ones", [1, 128], f32, kind="ExternalInput")
        b_d = nc.dram_tensor("b", [1, DEPTH * SIZE], f32, kind="ExternalInput")
    out_d = nc.dram_tensor("out", [rows, SIZE], f32, kind="ExternalOutput")

    with tile.TileContext(nc) as tc:
        with (
            tc.tile_pool(name="const", bufs=1) as constp,
            tc.tile_pool(name="state", bufs=1) as statep,
            tc.tile_pool(name="wtp", bufs=2) as wtp,
            tc.tile_pool(name="maskp", bufs=mask_bufs) as maskp,
            tc.tile_pool(name="tmpf", bufs=tmp_bufs) as tmpf,
            tc.tile_pool(name="tmpb", bufs=tmp_bufs) as tmpb,
            tc.tile_pool(name="stp", bufs=st_bufs) as stp,
            tc.tile_pool(name="psA", bufs=2, space="PSUM") as psA,
            tc.tile_pool(name="psM", bufs=2, space="PSUM") as psM,
            tc.tile_pool(name="psT2", bufs=2, space="PSUM") as psT2,
        ):
            id_sb = constp.tile([128, 128], f32, tag="id")
            nc.sync.dma_start(id_sb[:], id_d.ap())
            if abs_on_sc:
                biasA = constp.tile([128, 1], f32, tag="biasA")
                nc.vector.memset(biasA[:], -2.5)
                biasB = constp.tile([128, 1], f32, tag="biasB")
                nc.vector.memset(biasB[:], -6.0)
            if t_on_pe:
                ct_sb = constp.tile([128, 130], f32, tag="ct")
                nc.sync.dma_start(ct_sb[:], ct_d.ap())
            if with_bias:
                ones_sb = constp.tile([1, 128], f32, tag="ones")
                nc.sync.dma_start(ones_sb[:], ones_d.ap())
                b_sb = constp.tile([1, DEPTH * SIZE], f32, tag="b")
                nc.sync.dma_start(b_sb[:], b_d.ap())

            x_ap = x_d.ap()
            wt_ap = None if mm_pair else wt_d.ap()
            out_ap = out_d.ap()

            sts = [statep.tile([128, G, SIZE + 2], f32, tag=f"st{g}",
                               name=f"st{g}")
                   for g in range(ngroups)]

            for rep in range(repeat):
                for g in range(ngroups):
                    st = sts[g]
                    for i in range(G):
                        r0 = (g * G + i) * 128
                        nc.sync.dma_start(st[:, i, 1:SIZE + 1],
                                          x_ap[r0:r0 + 128, :])
                    nc.vector.tensor_copy(st[:, :, 0:1], st[:, :, SIZE:SIZE + 1])
                    nc.vector.tensor_copy(st[:, :, SIZE + 1:SIZE + 2],
                                          st[:, :, 1:2])
                if skip_mm:
                    mask_const = maskp.tile([128, G, SIZE], f32, tag="maskc")
                    nc.vector.memset(mask_const[:], 0.5)

                for d in range(DEPTH):
                    if not skip_mm:
                        if mm_pair:
                            wth_sb = wtp.tile([128, 4, SIZE], bf16, tag="wth")
                            nc.sync.dma_start(wth_sb[:], wth_d.ap()[:, d, :, :])
                            wtl_sb = wtp.tile([128, 4, SIZE], bf16, tag="wtl")
                            nc.sync.dma_start(wtl_sb[:], wtl_d.ap()[:, d, :, :])
                        else:
                            wt_sb = wtp.tile([128, 4, SIZE], f32, tag="wt")
                            nc.sync.dma_start(wt_sb[:], wt_ap[:, d, :, :])
                    for g in range(ngroups):
                        st = sts[g]
                        if skip_mm:
                            mask = mask_const
                        else:
                            pM = psM.tile([128, G, SIZE], f32, tag="pM")
                            mask = maskp.tile([128, G, SIZE], f32, tag="mask")
                        if t_on_pe:
                            g4 = tmpb.tile([128, G, SIZE], bf16, tag="g4")
                            g7 = tmpb.tile([128, G, SIZE], bf16, tag="g7")
                        for i in range(G if not skip_mm else 0):
                            pT = psA.tile([128, SIZE], f32, tag="pT")
                            for j in range(4):
                                nc.tensor.transpose(
                                    pT[:, j * 128:(j + 1) * 128],
                                    st[:, i, 1 + j * 128:1 + (j + 1) * 128],
                                    id_sb[:],
                                )
                            if mm_pair:
                                sTh = stp.tile([128, SIZE], bf16, tag="sTh")
                                nc.scalar.copy(sTh[:], pT[:])
                                sTl = stp.tile([128, SIZE], bf16, tag="sTl")
                                nc.vector.tensor_tensor(sTl[:], pT[:], sTh[:],
                                                        AL.subtract)
                                for j in range(4):
                                    cj = slice(j * 128, (j + 1) * 128)
                                    nc.tensor.matmul(
                                        pM[:, i, :], sTh[:, cj],
                                        wth_sb[:, j, :],
                                        start=(j == 0), stop=False)
                                    nc.tensor.matmul(
                                        pM[:, i, :], sTh[:, cj],
                                        wtl_sb[:, j, :],
                                        start=False, stop=False)
                                    nc.tensor.matmul(
                                        pM[:, i, :], sTl[:, cj],
                                        wth_sb[:, j, :],
                                        start=False,
                                        stop=(j == 3 and not with_bias))
                            else:
                                sT = stp.tile([128, SIZE], f32, tag="sT")
                                nc.scalar.copy(sT[:], pT[:])
                                for j in range(4):
                                    nc.tensor.matmul(
                                        pM[:, i, :],
                                        sT[:, j * 128:(j + 1) * 128],
                                        wt_sb[:, j, :],
                                        start=(j == 0),
                                        stop=(j == 3 and not with_bias),
                                    )
                            if with_bias:
                                nc.tensor.matmul(
                                    pM[:, i, :],
                                    ones_sb[0:1, :],
                                    b_sb[0:1, d * SIZE:(d + 1) * SIZE],
                                    start=False,
                                    stop=True,
                                )
                            if t_on_pe:
                                # banded circulant t = 4l + 2c + r on PE:
                                # chunk j owns out cols [128j, 128j+127];
                                # cross-chunk neighbor contributions are
                                # added by 1-col accumulate fixups.
                                pT2 = psT2.tile([128, SIZE], f32, tag="pT2")
                                for j in range(4):
                                    nc.tensor.matmul(
                                        pT2[:, j * 128:(j + 1) * 128],
                                        sT[:, j * 128:(j + 1) * 128],
                                        ct_sb[:, 1:129],
                                        start=(j == 0), stop=False,
                                        skip_group_check=True)
                                for j in range(4):
                                    ca = (j * 128 + 128) % SIZE
                                    nc.tensor.matmul(
                                        pT2[:, ca:ca + 1],
                                        sT[:, j * 128:(j + 1) * 128],
                                        ct_sb[:, 129:130],
                                        start=False, stop=False,
                                        skip_group_check=True)
                                    cb = (j * 128 + SIZE - 1) % SIZE
                                    nc.tensor.matmul(
                                        pT2[:, cb:cb + 1],
                                        sT[:, j * 128:(j + 1) * 128],
                                        ct_sb[:, 0:1],
                                        start=False, stop=(j == 3),
                                        skip_group_check=True)
                                # per-tile compares straight from PSUM
                                nc.vector.tensor_scalar(
                                    g4[:, i, :], pT2[:], 4.0, None, AL.is_ge)
                                nc.vector.tensor_scalar(
                                    g7[:, i, :], pT2[:], 7.0, None, AL.is_ge)
                                nc.vector.scalar_tensor_tensor(
                                    g4[:, i, :], pT2[:], 1.0, g4[:, i, :],
                                    AL.is_ge, AL.subtract)
                                nc.vector.scalar_tensor_tensor(
                                    g7[:, i, :], pT2[:], 5.0, g7[:, i, :],
                                    AL.is_ge, AL.subtract)
                        if not skip_mm:
                            nc.scalar.activation(mask[:], pM[:], AF.Sigmoid)
                        if skip_ew:
                            continue

                        l_ap = st[:, :, 0:SIZE]
                        c_ap = st[:, :, 1:SIZE + 1]
                        r_ap = st[:, :, 2:SIZE + 2]
                        tb = tmpf.tile([128, G, SIZE], f32, tag="tb")
                        if not t_on_pe:
                            c2 = tmpf.tile([128, G, SIZE], f32, tag="c2")
                            # u = 4*l + 2*c (both scalings exact, one add)
                            if c2_on_sc:
                                nc.scalar.mul(c2[:], c_ap, 2.0)
                            else:
                                nc.vector.tensor_scalar(c2[:], c_ap, 2.0, None,
                                                        AL.mult)
                            nc.vector.scalar_tensor_tensor(
                                c2[:], l_ap, 4.0, c2[:], AL.mult, AL.add)
                            # t = u + r
                            nc.vector.tensor_tensor(tb[:], c2[:], r_ap, AL.add)
                            t_src = tb[:]
                            g4 = tmpb.tile([128, G, SIZE], bf16, tag="g4")
                            g7 = tmpb.tile([128, G, SIZE], bf16, tag="g7")
                            if abs_on_sc:
                                # bits=1 iff t in [1,4) u [5,7):
                                # r1=|t-2.5|<1.5, r2=|t-6|<1 (exact-integer
                                # edge cases are measure-zero and accepted)
                                ra = tmpf.tile([128, G, SIZE], f32, tag="ra")
                                rb = tmpf.tile([128, G, SIZE], f32, tag="rb")
                                nc.scalar.activation(ra[:], t_src, AF.Abs,
                                                     bias=biasA[:])
                                nc.scalar.activation(rb[:], t_src, AF.Abs,
                                                     bias=biasB[:])
                                nc.vector.tensor_scalar(g4[:], ra[:], 1.5,
                                                        None, AL.is_lt)
                                nc.vector.tensor_scalar(g7[:], rb[:], 1.0,
                                                        None, AL.is_lt)
                            else:
                                nc.vector.tensor_scalar(g4[:], t_src, 4.0,
                                                        None, AL.is_ge)
                                nc.vector.tensor_scalar(g7[:], t_src, 7.0,
                                                        None, AL.is_ge)
                                # d1 = [t>=1]-[t>=4]; d2 = [t>=5]-[t>=7]
                                nc.vector.scalar_tensor_tensor(
                                    g4[:], t_src, 1.0, g4[:], AL.is_ge,
                                    AL.subtract)
                                nc.vector.scalar_tensor_tensor(
                                    g7[:], t_src, 5.0, g7[:], AL.is_ge,
                                    AL.subtract)
                        # bits = d1 + d2  (exact 0/1 in bf16)
                        nc.vector.tensor_tensor(g4[:], g4[:], g7[:], AL.add)
                        # e = bits - state
                        nc.vector.scalar_tensor_tensor(
                            tb[:], c_ap, -1.0, g4[:], AL.mult, AL.add)
                        # f = mask * e
                        nc.vector.tensor_tensor(tb[:], mask[:], tb[:], AL.mult)
                        # state += f   (in-place)
                        nc.vector.tensor_tensor(st[:, :, 1:SIZE + 1], tb[:],
                                                c_ap, AL.add)
                        # refresh circular-wrap pad columns
                        nc.vector.tensor_copy(st[:, :, 0:1],
                                              st[:, :, SIZE:SIZE + 1])
                        nc.vector.tensor_copy(st[:, :, SIZE + 1:SIZE + 2],
                                              st[:, :, 1:2])

                for g in range(ngroups):
                    st = sts[g]
                    for i in range(G):
                        r0 = (g * G + i) * 128
                        nc.sync.dma_start(out_ap[r0:r0 + 128, :],
                                          st[:, i, 1:SIZE + 1])

    nc.compile()
    return nc


def _host_inputs(W, b, with_bias, t_on_pe, mm_pair=False):
    import ml_dtypes

    W = np.asarray(W, dtype=np.float32)
    # wt[p, d, j, n] = W[d][n, j*128+p]
    wt = np.ascontiguousarray(
        W.transpose(0, 2, 1).reshape(DEPTH, 4, 128, SIZE).transpose(2, 0, 1, 3))
    common = {
        "ident": np.eye(128, dtype=np.float32),
    }
    if mm_pair:
        wt_hi = wt.astype(ml_dtypes.bfloat16)
        wt_lo = (wt - wt_hi.astype(np.float32)).astype(ml_dtypes.bfloat16)
        common["wt_hi"] = wt_hi
        common["wt_lo"] = wt_lo
    else:
        common["wt"] = wt
    if t_on_pe:
        # ct[k, m]: coeff of s_k in window col m: t_n = 4 s_{n-1} + 2 s_n
        # + s_{n+1}; main matmuls use ct[:,1:129], fixups cols 0 and 129.
        ct = np.zeros((128, 130), dtype=np.float32)
        for k in range(128):
            ct[k, k] = 1.0
            ct[k, k + 1] = 2.0
            ct[k, k + 2] = 4.0
        common["ct"] = ct
    if with_bias:
        common["ones"] = np.ones((1, 128), dtype=np.float32)
        common["b"] = np.ascontiguousarray(
            np.asarray(b, dtype=np.float32).reshape(1, DEPTH * SIZE))
    return common


# default configuration used by kernel(): depth-outer schedule, 3-term
# bf16-pair mask matmul, |t-c| bits layer on ScalarE, 2*c on ScalarE.
CFG = dict(G=2, t_on_pe=False, mm_pair=True, abs_on_sc=True, c2_on_sc=True)


def get_nc(with_bias, cfg=None):
    cfg = dict(CFG if cfg is None else cfg)
    key = (NTILES, with_bias, tuple(sorted(cfg.items())))
    if key not in _NC_CACHE:
        _NC_CACHE[key] = build_nc(NTILES, with_bias=with_bias, **cfg)
    return _NC_CACHE[key]


def make_in_maps(x, W, b, with_bias, cfg=None):
    cfg = dict(CFG if cfg is None else cfg)
    common = _host_inputs(W, b, with_bias, cfg.get("t_on_pe", False),
                          cfg.get("mm_pair", False))
    shards = np.asarray(x, dtype=np.float32).reshape(-1, ROWS_PER_CORE, SIZE)
    return [dict(common, x=np.ascontiguousarray(shards[i]))
            for i in range(shards.shape[0])]


def kernel(x, W, b):
    from concourse import bass_utils

    x = np.asarray(x, dtype=np.float32)
    b = np.asarray(b, dtype=np.float32)
    assert x.shape == (BATCH, SIZE)
    with_bias = bool(np.any(b))
    nc = get_nc(with_bias)
    in_maps = make_in_maps(x, W, b, with_bias)
    res = bass_utils.run_bass_kernel_spmd(nc, in_maps,
                                          core_ids=list(range(N_CORES)))
    out = np.concatenate([res.results[i]["out"] for i in range(N_CORES)], axis=0)
    return out.astype(np.float32, copy=False)
